# revision 1
# baseline (speedup 1.0000x reference)
"""BiGRU encoder kernel for 8 Trainium2 NeuronCores.

Strategy:
  - Reformulate the per-sample ragged windows as masked GRUs over FIXED
    position ranges: forward runs positions 0..7 ascending, backward runs
    positions 14..7 descending.  A sample with forward length lf only starts
    updating at position 8-lf; before that its hidden state must stay 0.
    That is enforced exactly by adding +BIG to the z-gate pre-activation for
    pre-start steps (z==1.0 => h' = n + z*(h-n) = n + (0-n) = 0 exactly).
  - Sort samples by window_len, deal them round-robin to the 8 cores (data
    parallel, near-identical length distribution per core).  Per core, two
    batch tiles of 512 samples; each GRU step runs only on the suffix of
    samples that are long enough to need it (suffix clamped to >=256 so
    float32r matmuls stay at full rate; over-included samples are exact via
    the z-mask and h-prefix memsets).
  - Everything on-device is computed in transposed (feature-major) layout:
    features on SBUF partitions, samples on the free dim, so the recurrence
    needs no runtime transposes.  Weights are transposed host-side.
  - Matmuls run as float32r (full-rate fp32 mode of the PE array).
"""

import os
from contextlib import ExitStack

import numpy as np

import concourse.bacc as bacc
import concourse.tile as tile
from concourse import mybir
from concourse.bass_utils import run_bass_kernel_spmd
from concourse.masks import make_identity

NCORES = 8
B, T, D, H = 8192, 15, 512, 512
G = 3 * H  # gate rows (r, z, n)
BIG = 40.0
S = 512  # samples per batch tile
F32 = mybir.dt.float32
DT_MM = mybir.dt.float32 if os.environ.get("GRU_DT") == "f32" else mybir.dt.float32r
H_ENGINE = os.environ.get("GRU_HUPD", "vector")  # engine for h-update chain

ACT = mybir.ActivationFunctionType
ALU = mybir.AluOpType

_PROGRAM_CACHE = {}
LAST_RESULT = None


def _build_program(sched):
    """sched: per tile, (f_steps, b_steps); each step = (width, masked)."""
    ntiles = len(sched)
    Bc = S * ntiles
    nc = bacc.Bacc("TRN2", target_bir_lowering=False, debug=False,
                   num_devices=NCORES)

    xT_d = nc.dram_tensor("xT", [T, D, Bc], DT_MM, kind="ExternalInput")
    wf_d = nc.dram_tensor("wf", [D + H, G], DT_MM, kind="ExternalInput")
    wb_d = nc.dram_tensor("wb", [D + H, G], DT_MM, kind="ExternalInput")
    w1_d = nc.dram_tensor("w1", [2 * H, H], DT_MM, kind="ExternalInput")
    w2_d = nc.dram_tensor("w2", [H, H], DT_MM, kind="ExternalInput")
    bias_d = nc.dram_tensor("bias", [40, 128], F32, kind="ExternalInput")
    mf_d = nc.dram_tensor("maskzf", [8, Bc], F32, kind="ExternalInput")
    mb_d = nc.dram_tensor("maskzb", [8, Bc], F32, kind="ExternalInput")
    y_d = nc.dram_tensor("y", [Bc, H], F32, kind="ExternalOutput")

    with tile.TileContext(nc) as tc, ExitStack() as ctx:
        const = ctx.enter_context(tc.tile_pool(name="const", bufs=1))
        wpool = ctx.enter_context(tc.tile_pool(name="w", bufs=2))
        xpool = ctx.enter_context(tc.tile_pool(name="x", bufs=2))
        hpool = ctx.enter_context(tc.tile_pool(name="h", bufs=2))
        hfin = ctx.enter_context(tc.tile_pool(name="hfin", bufs=4))
        gpool = ctx.enter_context(tc.tile_pool(name="g", bufs=5))
        mpool = ctx.enter_context(tc.tile_pool(name="m", bufs=1))
        opool = ctx.enter_context(tc.tile_pool(name="o", bufs=4))
        rzps = ctx.enter_context(tc.tile_pool(name="rz", bufs=4, space="PSUM"))
        xpps = ctx.enter_context(tc.tile_pool(name="xp", bufs=2, space="PSUM"))
        ghps = ctx.enter_context(tc.tile_pool(name="gh", bufs=2, space="PSUM"))

        # Weights [128, kchunk, gate-cols]; kchunks 0-3 input dims, 4-7 hidden
        # dims.  wf/wb/w1 time-share a 2-slot pool (one tag); per-kchunk DMAs
        # so the first matmuls start as soon as chunk 0 lands.
        def load_w(dram, kchunks, cols, name, pool=None, sync_chunks=()):
            t_ = (pool or wpool).tile([128, kchunks, cols], DT_MM,
                                      tag="w" if pool is None else "const",
                                      name=name)
            src = dram.rearrange("(c k) g -> k c g", k=128)
            for c in range(kchunks):
                eng = nc.sync if c in sync_chunks else nc.scalar
                eng.dma_start(t_[:, c, :], src[:, c, :])
            return t_

        wf = load_w(wf_d, 8, G, "wf", sync_chunks=(0, 1, 2, 3))
        wb = load_w(wb_d, 8, G, "wb")
        w2 = load_w(w2_d, 4, H, "w2", pool=const)
        bt = const.tile([128, 40], F32)
        nc.gpsimd.dma_start(bt[:], bias_d.rearrange("n p -> p n"))
        ident = const.tile([128, 128], F32)
        make_identity(nc, ident[:])

        heng = nc.gpsimd if H_ENGINE == "gpsimd" else nc.vector

        def emit_dir(s0, steps, w, mask_d, bb, pos_fn):
            """One GRU direction over one batch tile; returns final h tile."""
            nsteps = len(steps)
            h_prev = None
            for j, (width, masked) in enumerate(steps):
                first = j == 0
                p_abs = pos_fn(j)
                so = S - width  # suffix offset within the tile
                a0 = s0 + so
                xt = xpool.tile([128, 4, S], DT_MM, tag="x", name="xt")
                nc.sync.dma_start(
                    xt[:, :, :width],
                    xT_d[p_abs].rearrange("(c k) s -> k c s", k=128)[:, :, a0:s0 + S],
                )
                mt = None
                if masked:
                    mt = mpool.tile([128, S], F32, tag="m", name="mt")
                    nc.gpsimd.dma_start(
                        mt[:, :width],
                        mask_d[8 - nsteps + j, a0:s0 + S].partition_broadcast(128),
                    )
                h_next = (hfin if j == nsteps - 1 else hpool).tile(
                    [128, 4, S], DT_MM, tag="hf" if j == nsteps - 1 else "h",
                    name="h")
                if j + 1 < nsteps:
                    nso = S - steps[j + 1][0]  # next step's suffix offset
                    if nso < so:
                        nc.gpsimd.memset(h_next[:, :, nso:so].bitcast(F32), 0.0)

                rps, zps, xpns, ghns = [], [], [], []
                for i in range(4):
                    # separate PSUM tiles per accumulation group: start=True
                    # clears the whole bank, so groups must not share one
                    r_ps = rzps.tile([128, width], F32, tag="rz", name=f"rps{i}")
                    z_ps = rzps.tile([128, width], F32, tag="rz", name=f"zps{i}")
                    xpn = xpps.tile([128, width], F32, tag="xp", name=f"xpn{i}")
                    rps.append(r_ps)
                    zps.append(z_ps)
                    xpns.append(xpn)
                    for k in range(4):
                        st = k == 0
                        sp_rz = first and k == 3
                        xk = xt[:, k, :width]
                        nc.tensor.matmul(r_ps[:], w[:, k, i * 128:(i + 1) * 128],
                                         xk, start=st, stop=sp_rz)
                        nc.tensor.matmul(z_ps[:],
                                         w[:, k, H + i * 128:H + (i + 1) * 128],
                                         xk, start=st, stop=sp_rz)
                        nc.tensor.matmul(xpn[:],
                                         w[:, k, 2 * H + i * 128:2 * H + (i + 1) * 128],
                                         xk, start=st, stop=k == 3)
                if not first:
                    for i in range(4):
                        ghn = ghps.tile([128, width], F32, tag="gh", name=f"ghn{i}")
                        ghns.append(ghn)
                        for k in range(4):
                            hk = h_prev[:, k, so:]
                            nc.tensor.matmul(rps[i][:],
                                             w[:, 4 + k, i * 128:(i + 1) * 128],
                                             hk, start=False, stop=k == 3)
                            nc.tensor.matmul(zps[i][:],
                                             w[:, 4 + k, H + i * 128:H + (i + 1) * 128],
                                             hk, start=False, stop=k == 3)
                            nc.tensor.matmul(ghn[:],
                                             w[:, 4 + k, 2 * H + i * 128:2 * H + (i + 1) * 128],
                                             hk, start=k == 0, stop=k == 3)

                for i in range(4):
                    xpn = xpns[i]
                    r = gpool.tile([128, width], F32, tag="g", name="r")
                    nc.scalar.activation(r[:], rps[i][:], ACT.Sigmoid,
                                         bias=bt[:, bb + i:bb + i + 1])
                    if masked:
                        zin = gpool.tile([128, width], F32, tag="g", name="zin")
                        nc.vector.tensor_add(zin[:], zps[i][:], mt[:, :width])
                        zsrc = zin[:]
                    else:
                        zsrc = zps[i][:]
                    z = gpool.tile([128, width], F32, tag="g", name="z")
                    nc.scalar.activation(z[:], zsrc, ACT.Sigmoid,
                                         bias=bt[:, bb + 4 + i:bb + 5 + i])
                    tt = gpool.tile([128, width], F32, tag="g", name="tt")
                    if first:
                        nc.vector.tensor_scalar_mul(tt[:], r[:],
                                                    bt[:, bb + 8 + i:bb + 9 + i])
                    else:
                        nc.vector.scalar_tensor_tensor(
                            tt[:], ghns[i][:], bt[:, bb + 8 + i:bb + 9 + i], r[:],
                            op0=ALU.add, op1=ALU.mult)
                    ss = gpool.tile([128, width], F32, tag="g", name="ss")
                    nc.vector.tensor_add(ss[:], tt[:], xpn[:])
                    n = gpool.tile([128, width], F32, tag="g", name="n")
                    nc.scalar.activation(n[:], ss[:], ACT.Tanh,
                                         bias=bt[:, bb + 12 + i:bb + 13 + i])
                    ho = h_next[:, i, so:]
                    if first:
                        e = gpool.tile([128, width], F32, tag="g", name="e")
                        heng.tensor_mul(e[:], z[:], n[:])
                        heng.tensor_sub(ho, n[:], e[:])
                    else:
                        dd = gpool.tile([128, width], F32, tag="g", name="dd")
                        heng.tensor_sub(dd[:], h_prev[:, i, so:], n[:])
                        e = gpool.tile([128, width], F32, tag="g", name="e")
                        heng.tensor_mul(e[:], z[:], dd[:])
                        heng.tensor_add(ho, n[:], e[:])
                h_prev = h_next
            return h_prev

        hfs = []
        for t in range(ntiles):
            nf = len(sched[t][0])
            hfs.append(emit_dir(t * S, sched[t][0], wf, mf_d, 0,
                                lambda j, nf=nf: 8 - nf + j))
        w1 = load_w(w1_d, 8, H, "w1")

        def emit_mlp(t, hf, hb):
            hid = []
            for i in range(4):
                ps = xpps.tile([128, S], F32, tag="xp", name="mps")
                for k in range(8):
                    src = hf if k < 4 else hb
                    nc.tensor.matmul(ps[:], w1[:, k, i * 128:(i + 1) * 128],
                                     src[:, k % 4, :], start=k == 0, stop=k == 7)
                h32 = gpool.tile([128, S], F32, tag="g", name="h32")
                nc.scalar.activation(h32[:], ps[:], ACT.Relu,
                                     bias=bt[:, 32 + i:33 + i])
                hr = gpool.tile([128, S], DT_MM, tag="g", name="hr")
                nc.vector.tensor_copy(hr[:], h32[:])
                hid.append(hr)
            onats = []
            for gidx in range(S // 128):
                onat = opool.tile([128, H], F32, tag="o", name=f"onat{gidx}")
                onats.append(onat)
            for i in range(4):
                ps = xpps.tile([128, S], F32, tag="xp", name="ops")
                for k in range(4):
                    nc.tensor.matmul(ps[:], w2[:, k, i * 128:(i + 1) * 128],
                                     hid[k][:], start=k == 0, stop=k == 3)
                o32 = gpool.tile([128, S], F32, tag="g", name="o32")
                nc.vector.tensor_scalar_add(o32[:], ps[:], bt[:, 36 + i:37 + i])
                for gidx in range(S // 128):
                    tp = ghps.tile([128, 128], F32, tag="gh", name="tp")
                    nc.tensor.transpose(tp[:], o32[:, gidx * 128:(gidx + 1) * 128],
                                        ident[:])
                    nc.vector.tensor_copy(onats[gidx][:, i * 128:(i + 1) * 128],
                                          tp[:])
            for gidx in range(S // 128):
                r0 = t * S + gidx * 128
                nc.sync.dma_start(y_d[r0:r0 + 128, :], onats[gidx][:])

        for t in range(ntiles):
            nb = len(sched[t][1])
            hb = emit_dir(t * S, sched[t][1], wb, mb_d, 16,
                          lambda j, nb=nb: 6 + nb - j)
            emit_mlp(t, hfs[t], hb)

    nc.compile()
    return nc


def kernel(padded_window, window_len, Wih_f, Whh_f, bih_f, bhh_f,
           Wih_b, Whh_b, bih_b, bhh_b, W1, b1, W2, b2):
    wl = np.asarray(window_len)
    lf = (wl - 1) // 2 + 1
    lb = wl // 2 + 1
    order = np.argsort(wl, kind="stable")

    Bc = B // NCORES
    ntiles = Bc // S
    # per-core sorted lengths: row k = per-core rank k, column = core
    lf_pc = lf[order].reshape(-1, NCORES)
    lb_pc = lb[order].reshape(-1, NCORES)

    def dir_steps(lens_pc, t):
        seg = lens_pc[t * S:(t + 1) * S]  # [S, NCORES]
        n = int(seg.max())
        steps = []
        for j in range(n):
            need = n - j
            cnt = (seg >= need).sum(axis=0)  # samples needing this step, per core
            w = int(min(S, max(256, -(-int(cnt.max()) // 64) * 64)))
            masked = bool(cnt.min() < w)
            steps.append((w, masked))
        return tuple(steps)

    sched = tuple((dir_steps(lf_pc, t), dir_steps(lb_pc, t))
                  for t in range(ntiles))

    if sched not in _PROGRAM_CACHE:
        _PROGRAM_CACHE[sched] = _build_program(sched)
    nc = _PROGRAM_CACHE[sched]

    f32 = np.float32
    wf = np.concatenate([Wih_f.T, Whh_f.T], 0).astype(f32)
    wb = np.concatenate([Wih_b.T, Whh_b.T], 0).astype(f32)
    w1 = np.ascontiguousarray(W1.T, dtype=f32)
    w2 = np.ascontiguousarray(W2.T, dtype=f32)

    def chunks(v):  # [512] -> [4, 128]
        return np.asarray(v, f32).reshape(4, 128)

    bias = np.concatenate([
        chunks((bih_f + bhh_f)[:H]), chunks((bih_f + bhh_f)[H:2 * H]),
        chunks(bhh_f[2 * H:]), chunks(bih_f[2 * H:]),
        chunks((bih_b + bhh_b)[:H]), chunks((bih_b + bhh_b)[H:2 * H]),
        chunks(bhh_b[2 * H:]), chunks(bih_b[2 * H:]),
        chunks(b1), chunks(b2),
    ], 0)  # [40, 128]

    pw = np.asarray(padded_window, f32)
    in_maps = []
    p8 = np.arange(8)
    for c in range(NCORES):
        idx = order[c::NCORES]
        xT = np.ascontiguousarray(pw[idx].transpose(1, 2, 0))  # [15, 512, Bc]
        mzf = (BIG * (p8[:, None] < (8 - lf[idx])[None, :])).astype(f32)
        mzb = (BIG * (p8[:, None] < (8 - lb[idx])[None, :])).astype(f32)
        in_maps.append({
            "xT": xT, "wf": wf, "wb": wb, "w1": w1, "w2": w2,
            "bias": bias, "maskzf": mzf, "maskzb": mzb,
        })

    trace = bool(os.environ.get("GRU_TRACE"))
    kw = {}
    if os.environ.get("GRU_TMPDIR"):
        kw["tmpdir"] = os.environ["GRU_TMPDIR"]
    res = run_bass_kernel_spmd(nc, in_maps, core_ids=list(range(NCORES)),
                               trace=trace, **kw)
    global LAST_RESULT
    LAST_RESULT = res
    out = np.empty((B, H), f32)
    for c in range(NCORES):
        out[order[c::NCORES]] = res.results[c]["y"]
    return out



# revision 4
# speedup vs baseline: 1.1473x; 1.1473x over previous
"""BiGRU encoder kernel for 8 Trainium2 NeuronCores.

Strategy (v2, fp8 DoubleRow):
  - Masked GRU over FIXED position ranges as before: forward runs positions
    (8-n)..7 ascending, backward (6+n)..7 descending; a sample of length l
    starts at step n-l with h=0 (prefix memset) and a +BIG z-gate mask keeps
    over-included samples at exactly h=0 until their true start.
  - Sort samples by window_len, deal round-robin to 8 cores; per core two
    batch tiles of 512 sorted samples.  Step widths are EXACT per-step active
    counts (max over cores), rounded up to 16 only so SBUF suffix offsets stay
    16B-aligned; the <=15+spread over-included samples are fixed by a narrow
    z-mask add.
  - Matmuls: fp8e4 DoubleRow (K=256 per instruction, measured 2x throughput)
    for ALL hidden projections and for input projections except the last
    N_HI=3 steps of each stream, which run in bf16 for accuracy (fp8 error on
    late steps flows undamped into the output).  Weights are pre-scaled by
    512 (exact power-of-2) so unscaled fp8 x/h stay in e4m3's normal range;
    every activation rescales with scale=1/512.
  - h is carried in bf16 (fp8 carry compounds error); an fp8 copy for the
    next step's matmul operand is produced by a parallel gpsimd op.
  - Hidden projections run at the PREVIOUS step's width (newly exposed
    samples have h=0 so contribute nothing); the n-gate pre-activation is
    assembled with a split tensor op at the exposure boundary.
  - All four streams (2 tiles x fwd/bwd) are interleaved super-step by
    super-step so gate latency of one stream hides under matmuls of others.
  - Output is written feature-major [H, Bc]; the host transposes (free).
"""

import os
from contextlib import ExitStack

import numpy as np
import ml_dtypes

import concourse.bacc as bacc
import concourse.tile as tile
from concourse import mybir
from concourse.bass_utils import run_bass_kernel_spmd

NCORES = 8
B, T, D, H = 8192, 15, 512, 512
G = 3 * H
BIG = 40.0
S = 512
N_HI = int(os.environ.get("GRU_NHI", "3"))  # last-k steps with bf16 input proj
F32 = mybir.dt.float32
BF16 = mybir.dt.bfloat16
F8 = mybir.dt.float8e4
DR = mybir.MatmulPerfMode.DoubleRow

ACT = mybir.ActivationFunctionType
ALU = mybir.AluOpType

NP_BF = ml_dtypes.bfloat16
NP_F8 = ml_dtypes.float8_e4m3

_PROGRAM_CACHE = {}
LAST_RESULT = None


def _build_program(sched):
    """sched[t][d] = tuple of (w, w_prev_hidden, mw, hi) per step.
    w: step width (16-mult); mw: masked prefix width; hi: bf16 input proj."""
    ntiles = len(sched)
    Bc = S * ntiles
    nc = bacc.Bacc("TRN2", target_bir_lowering=False, debug=False,
                   num_devices=NCORES)

    x8_d = nc.dram_tensor("x8", [T, D, Bc], F8, kind="ExternalInput")
    xb_d = nc.dram_tensor("xb", [T, D, Bc], BF16, kind="ExternalInput")
    w8f_d = nc.dram_tensor("w8f", [D + H, G], F8, kind="ExternalInput")
    w8b_d = nc.dram_tensor("w8b", [D + H, G], F8, kind="ExternalInput")
    wbf_d = nc.dram_tensor("wbf", [D, G], BF16, kind="ExternalInput")
    wbb_d = nc.dram_tensor("wbb", [D, G], BF16, kind="ExternalInput")
    w1_d = nc.dram_tensor("w1", [2 * H, H], BF16, kind="ExternalInput")
    w2_d = nc.dram_tensor("w2", [H, H], BF16, kind="ExternalInput")
    bias_d = nc.dram_tensor("bias", [40, 128], F32, kind="ExternalInput")
    mf_d = nc.dram_tensor("maskzf", [8, Bc], BF16, kind="ExternalInput")
    mb_d = nc.dram_tensor("maskzb", [8, Bc], BF16, kind="ExternalInput")
    y_d = nc.dram_tensor("y", [H, Bc], F32, kind="ExternalOutput")

    with tile.TileContext(nc) as tc, ExitStack() as ctx:
        const = ctx.enter_context(tc.tile_pool(name="const", bufs=1))
        x8pool = ctx.enter_context(tc.tile_pool(name="x8", bufs=4))
        xbpool = ctx.enter_context(tc.tile_pool(name="xb", bufs=4))
        hbf = [ctx.enter_context(tc.tile_pool(name=f"hb{s}", bufs=2))
               for s in range(4)]
        hf8 = [ctx.enter_context(tc.tile_pool(name=f"h8{s}", bufs=2))
               for s in range(4)]
        hfin = ctx.enter_context(tc.tile_pool(name="hfin", bufs=4))
        gpool = ctx.enter_context(tc.tile_pool(name="g", bufs=16))
        mpool = ctx.enter_context(tc.tile_pool(name="m", bufs=2))
        opool = ctx.enter_context(tc.tile_pool(name="o", bufs=2))
        rzps = ctx.enter_context(tc.tile_pool(name="rz", bufs=4, space="PSUM"))
        xpps = ctx.enter_context(tc.tile_pool(name="xp", bufs=2, space="PSUM"))
        ghps = ctx.enter_context(tc.tile_pool(name="gh", bufs=2, space="PSUM"))

        def load_w(dram, kchunks, cols, dt, name):
            t_ = const.tile([128, kchunks, cols], dt, name=name)
            src = dram.rearrange("(c k) g -> k c g", k=128)
            for c in range(kchunks):
                nc.scalar.dma_start(t_[:, c, :], src[:, c, :])
            return t_

        w8 = [load_w(w8f_d, 8, G, F8, "w8f"), load_w(w8b_d, 8, G, F8, "w8b")]
        wbf = [load_w(wbf_d, 4, G, BF16, "wbf"), load_w(wbb_d, 4, G, BF16, "wbb")]
        w1 = load_w(w1_d, 8, H, BF16, "w1")
        w2 = load_w(w2_d, 4, H, BF16, "w2")
        bt = const.tile([128, 40], F32)
        nc.gpsimd.dma_start(bt[:], bias_d.rearrange("n p -> p n"))
        mask_d = [mf_d, mb_d]

        # stream state: (h_prev_bf, h_prev_f8, prev_w)
        state = {}

        def emit_step(t, d, j, steps):
            w, wh, mw, hi = steps[j]
            n = len(steps)
            first = j == 0
            last = j == n - 1
            so = S - w
            soh = S - wh if not first else None  # hidden-proj suffix offset
            a0 = t * S + so
            bb = 16 * d
            pos = (8 - n + j) if d == 0 else (6 + n - j)
            skey = 2 * t + d

            if hi:
                xt = xbpool.tile([128, 4, S], BF16, tag="xb", name="xt")
                nc.sync.dma_start(
                    xt[:, :, so:],
                    xb_d[pos].rearrange("(c k) s -> k c s", k=128)[:, :, a0:a0 + w])
            else:
                xt = x8pool.tile([128, 4, S], F8, tag="x8", name="xt")
                nc.sync.dma_start(
                    xt[:, :, so:],
                    x8_d[pos].rearrange("(c k) s -> k c s", k=128)[:, :, a0:a0 + w])
            mt = None
            if mw:
                mt = mpool.tile([128, 64], BF16, tag="m", name="mt")
                nc.gpsimd.dma_start(
                    mt[:, :mw],
                    mask_d[d][8 - (n - j), a0:a0 + mw].partition_broadcast(128))

            if first:
                h_prev = h8_prev = None
            else:
                h_prev, h8_prev, _ = state[skey]
            hb_next = (hfin if last else hbf[skey]).tile(
                [128, 4, S], BF16, tag="hf" if last else f"h{skey}", name="hb")
            h8_next = None
            if not last:
                h8_next = hf8[skey].tile([128, 4, S], F8, tag=f"g8{skey}",
                                         name="h8")
                nw = steps[j + 1][0]
                if S - nw < so:  # zero newly exposed prefix for next step's dd
                    nc.gpsimd.memset(hb_next[:, :, S - nw:so].bitcast(F32), 0.0)

            rps, zps, xpns, ghns = [], [], [], []
            for i in range(4):
                c0 = i * 128
                r_ps = rzps.tile([128, w], F32, tag="rz", name=f"rps{i}")
                z_ps = rzps.tile([128, w], F32, tag="rz", name=f"zps{i}")
                xpn = xpps.tile([128, w], F32, tag="xp", name=f"xpn{i}")
                rps.append(r_ps); zps.append(z_ps); xpns.append(xpn)
                # input projections; last k-chunk deferred until after hidden
                # so the full-width stop lands on a full-width instruction
                if hi:
                    ww = wbf[d]
                    for k in range(4):
                        st = k == 0
                        lastk = k == 3 and first
                        nc.tensor.matmul(r_ps[:], ww[:, k, c0:c0 + 128],
                                         xt[:, k, so:], start=st, stop=lastk)
                        nc.tensor.matmul(z_ps[:], ww[:, k, H + c0:H + c0 + 128],
                                         xt[:, k, so:], start=st, stop=lastk)
                        nc.tensor.matmul(xpn[:], ww[:, k, 2 * H + c0:2 * H + c0 + 128],
                                         xt[:, k, so:], start=st, stop=k == 3)
                else:
                    ww = w8[d]
                    for p in range(2):
                        st = p == 0
                        lastk = p == 1 and first
                        ksl = slice(2 * p, 2 * p + 2)
                        nc.tensor.matmul(r_ps[:], ww[:, ksl, c0:c0 + 128],
                                         xt[:, ksl, so:], start=st, stop=lastk,
                                         perf_mode=DR)
                        nc.tensor.matmul(z_ps[:], ww[:, ksl, H + c0:H + c0 + 128],
                                         xt[:, ksl, so:], start=st, stop=lastk,
                                         perf_mode=DR)
                        nc.tensor.matmul(xpn[:], ww[:, ksl, 2 * H + c0:2 * H + c0 + 128],
                                         xt[:, ksl, so:], start=st, stop=p == 1,
                                         perf_mode=DR)
                if not first:
                    wwh = w8[d]
                    ghn = ghps.tile([128, wh], F32, tag="gh", name=f"ghn{i}")
                    ghns.append(ghn)
                    for p in range(2):
                        ksl = slice(4 + 2 * p, 4 + 2 * p + 2)
                        nc.tensor.matmul(rps[i][:, soh - so:],
                                         wwh[:, ksl, c0:c0 + 128],
                                         h8_prev[:, 2 * p:2 * p + 2, soh:],
                                         start=False, stop=p == 1, perf_mode=DR)
                        nc.tensor.matmul(zps[i][:, soh - so:],
                                         wwh[:, ksl, H + c0:H + c0 + 128],
                                         h8_prev[:, 2 * p:2 * p + 2, soh:],
                                         start=False, stop=p == 1, perf_mode=DR)
                        nc.tensor.matmul(ghn[:],
                                         wwh[:, ksl, 2 * H + c0:2 * H + c0 + 128],
                                         h8_prev[:, 2 * p:2 * p + 2, soh:],
                                         start=p == 0, stop=p == 1, perf_mode=DR)

            for i in range(4):
                if mw:
                    nc.vector.tensor_add(zps[i][:, :mw], zps[i][:, :mw],
                                         mt[:, :mw])
                r = gpool.tile([128, w], BF16, tag="g", name="r")
                nc.scalar.activation(r[:], rps[i][:], ACT.Sigmoid,
                                     bias=bt[:, bb + i:bb + i + 1],
                                     scale=1.0 / 512)
                z = gpool.tile([128, w], BF16, tag="g", name="z")
                nc.scalar.activation(z[:], zps[i][:], ACT.Sigmoid,
                                     bias=bt[:, bb + 4 + i:bb + 5 + i],
                                     scale=1.0 / 512)
                tt = gpool.tile([128, w], BF16, tag="g", name="tt")
                if first:
                    nc.vector.tensor_scalar_mul(tt[:], r[:],
                                                bt[:, bb + 8 + i:bb + 9 + i])
                else:
                    dd = soh - so
                    if dd:
                        nc.vector.tensor_scalar_mul(tt[:, :dd], r[:, :dd],
                                                    bt[:, bb + 8 + i:bb + 9 + i])
                    nc.vector.scalar_tensor_tensor(
                        tt[:, dd:], ghns[i][:], bt[:, bb + 8 + i:bb + 9 + i],
                        r[:, dd:], op0=ALU.add, op1=ALU.mult)
                ss = gpool.tile([128, w], BF16, tag="g", name="ss")
                nc.vector.tensor_add(ss[:], tt[:], xpns[i][:])
                nn = gpool.tile([128, w], BF16, tag="g", name="n")
                nc.scalar.activation(nn[:], ss[:], ACT.Tanh,
                                     bias=bt[:, bb + 12 + i:bb + 13 + i],
                                     scale=1.0 / 512)
                ho = hb_next[:, i, so:]
                if first:
                    e = gpool.tile([128, w], BF16, tag="g", name="e")
                    nc.vector.tensor_mul(e[:], z[:], nn[:])
                    nc.vector.tensor_sub(ho, nn[:], e[:])
                    if h8_next is not None:
                        nc.gpsimd.tensor_sub(h8_next[:, i, so:], nn[:], e[:])
                else:
                    dd_t = gpool.tile([128, w], BF16, tag="g", name="dd")
                    nc.gpsimd.tensor_sub(dd_t[:], h_prev[:, i, so:], nn[:])
                    e = gpool.tile([128, w], BF16, tag="g", name="e")
                    nc.vector.tensor_mul(e[:], z[:], dd_t[:])
                    nc.vector.tensor_add(ho, nn[:], e[:])
                    if h8_next is not None:
                        nc.gpsimd.tensor_add(h8_next[:, i, so:], nn[:], e[:])
            state[skey] = (hb_next, h8_next, w)
            return hb_next

        def emit_mlp(t, hf_t, hb_t):
            hid = []
            for i in range(4):
                ps = xpps.tile([128, S], F32, tag="xp", name="mps")
                for k in range(8):
                    src = hf_t if k < 4 else hb_t
                    nc.tensor.matmul(ps[:], w1[:, k, i * 128:(i + 1) * 128],
                                     src[:, k % 4, :], start=k == 0, stop=k == 7)
                hr = gpool.tile([128, S], BF16, tag="g", name="hr")
                nc.scalar.activation(hr[:], ps[:], ACT.Relu,
                                     bias=bt[:, 32 + i:33 + i])
                hid.append(hr)
            for i in range(4):
                ps = xpps.tile([128, S], F32, tag="xp", name="ops")
                for k in range(4):
                    nc.tensor.matmul(ps[:], w2[:, k, i * 128:(i + 1) * 128],
                                     hid[k][:], start=k == 0, stop=k == 3)
                o32 = opool.tile([128, S], F32, tag="o", name="o32")
                nc.vector.tensor_scalar_add(o32[:], ps[:], bt[:, 36 + i:37 + i])
                nc.sync.dma_start(y_d[i * 128:(i + 1) * 128, t * S:(t + 1) * S],
                                  o32[:])

        # interleave the four streams super-step by super-step (starts aligned)
        nmax = max(len(sched[t][d]) for t in range(ntiles) for d in range(2))
        hfs = {}
        mlp_done = set()
        for J in range(nmax):
            for t in range(ntiles):
                for d in range(2):
                    steps = sched[t][d]
                    if J < len(steps):
                        h = emit_step(t, d, J, steps)
                        if J == len(steps) - 1:
                            hfs[(t, d)] = h
            for t in range(ntiles):
                if t not in mlp_done and (t, 0) in hfs and (t, 1) in hfs:
                    emit_mlp(t, hfs[(t, 0)], hfs[(t, 1)])
                    mlp_done.add(t)

    nc.compile()
    return nc


def _mk_sched(lens_pc, t):
    """lens_pc: [1024, NCORES] per-core sorted lengths; tile t rows."""
    seg = lens_pc[t * S:(t + 1) * S]  # [S, NCORES]
    n = int(seg.max())
    steps = []
    for j in range(n):
        need = n - j
        cnt = (seg >= need).sum(axis=0)
        w = min(S, -(-int(cnt.max()) // 16) * 16)
        mw = int(w - int(cnt.min()))
        hi = j >= n - N_HI
        steps.append([w, 0, mw, hi])
    for j in range(1, n):
        steps[j][1] = steps[j - 1][0]  # hidden width = prev step width
    return tuple(tuple(s) for s in steps)


def kernel(padded_window, window_len, Wih_f, Whh_f, bih_f, bhh_f,
           Wih_b, Whh_b, bih_b, bhh_b, W1, b1, W2, b2):
    wl = np.asarray(window_len)
    lf = (wl - 1) // 2 + 1
    lb = wl // 2 + 1
    order = np.argsort(wl, kind="stable")

    Bc = B // NCORES
    ntiles = Bc // S
    lf_pc = lf[order].reshape(-1, NCORES)
    lb_pc = lb[order].reshape(-1, NCORES)

    sched = tuple((_mk_sched(lf_pc, t), _mk_sched(lb_pc, t))
                  for t in range(ntiles))

    if sched not in _PROGRAM_CACHE:
        _PROGRAM_CACHE[sched] = _build_program(sched)
    nc = _PROGRAM_CACHE[sched]

    f32 = np.float32
    wf_full = np.concatenate([Wih_f.T, Whh_f.T], 0).astype(f32) * 512.0
    wb_full = np.concatenate([Wih_b.T, Whh_b.T], 0).astype(f32) * 512.0
    w8f = np.clip(wf_full, -240, 240).astype(NP_F8)
    w8b = np.clip(wb_full, -240, 240).astype(NP_F8)
    wbf = wf_full[:D].astype(NP_BF)
    wbb = wb_full[:D].astype(NP_BF)
    w1 = np.ascontiguousarray(W1.T, dtype=f32).astype(NP_BF)
    w2 = np.ascontiguousarray(W2.T, dtype=f32).astype(NP_BF)

    def chunks(v):
        return np.asarray(v, f32).reshape(4, 128)

    bias = np.concatenate([
        chunks((bih_f + bhh_f)[:H]), chunks((bih_f + bhh_f)[H:2 * H]),
        chunks(bhh_f[2 * H:] * 512.0), chunks(bih_f[2 * H:]),
        chunks((bih_b + bhh_b)[:H]), chunks((bih_b + bhh_b)[H:2 * H]),
        chunks(bhh_b[2 * H:] * 512.0), chunks(bih_b[2 * H:]),
        chunks(b1), chunks(b2),
    ], 0)  # [40, 128]

    pw = np.asarray(padded_window, f32)
    in_maps = []
    p8 = np.arange(8)
    for c in range(NCORES):
        idx = order[c::NCORES]
        xT = np.ascontiguousarray(pw[idx].transpose(1, 2, 0))  # [15, 512, Bc]
        mzf = (512.0 * BIG * (p8[:, None] < (8 - lf[idx])[None, :])).astype(NP_BF)
        mzb = (512.0 * BIG * (p8[:, None] < (8 - lb[idx])[None, :])).astype(NP_BF)
        in_maps.append({
            "x8": np.clip(xT, -240, 240).astype(NP_F8),
            "xb": xT.astype(NP_BF),
            "w8f": w8f, "w8b": w8b, "wbf": wbf, "wbb": wbb,
            "w1": w1, "w2": w2, "bias": bias,
            "maskzf": mzf, "maskzb": mzb,
        })

    trace = bool(os.environ.get("GRU_TRACE"))
    kw = {}
    if os.environ.get("GRU_TMPDIR"):
        kw["tmpdir"] = os.environ["GRU_TMPDIR"]
    res = run_bass_kernel_spmd(nc, in_maps, core_ids=list(range(NCORES)),
                               trace=trace, **kw)
    global LAST_RESULT
    LAST_RESULT = res
    out = np.empty((B, H), f32)
    for c in range(NCORES):
        out[order[c::NCORES]] = res.results[c]["y"].T
    return out


# revision 8
# speedup vs baseline: 1.3499x; 1.1766x over previous
"""BiGRU encoder kernel for 8 Trainium2 NeuronCores.

Strategy (v2, fp8 DoubleRow):
  - Masked GRU over FIXED position ranges as before: forward runs positions
    (8-n)..7 ascending, backward (6+n)..7 descending; a sample of length l
    starts at step n-l with h=0 (prefix memset) and a +BIG z-gate mask keeps
    over-included samples at exactly h=0 until their true start.
  - Sort samples by window_len, deal round-robin to 8 cores; per core two
    batch tiles of 512 sorted samples.  Step widths are EXACT per-step active
    counts (max over cores), rounded up to 16 only so SBUF suffix offsets stay
    16B-aligned; the <=15+spread over-included samples are fixed by a narrow
    z-mask add.
  - Matmuls: fp8e4 DoubleRow (K=256 per instruction, measured 2x throughput)
    for ALL hidden projections and for input projections except the last
    N_HI=3 steps of each stream, which run in bf16 for accuracy (fp8 error on
    late steps flows undamped into the output).  Weights are pre-scaled by
    512 (exact power-of-2) so unscaled fp8 x/h stay in e4m3's normal range;
    every activation rescales with scale=1/512.
  - h is carried in bf16 (fp8 carry compounds error); an fp8 copy for the
    next step's matmul operand is produced by a parallel gpsimd op.
  - Hidden projections run at the PREVIOUS step's width (newly exposed
    samples have h=0 so contribute nothing); the n-gate pre-activation is
    assembled with a split tensor op at the exposure boundary.
  - All four streams (2 tiles x fwd/bwd) are interleaved super-step by
    super-step so gate latency of one stream hides under matmuls of others.
  - Output is written feature-major [H, Bc]; the host transposes (free).
"""

import os
from contextlib import ExitStack

import numpy as np
import ml_dtypes

import concourse.bacc as bacc
import concourse.tile as tile
from concourse import mybir
from concourse.bass_utils import run_bass_kernel_spmd

NCORES = 8
B, T, D, H = 8192, 15, 512, 512
G = 3 * H
BIG = 40.0
S = 512
N_HI = int(os.environ.get("GRU_NHI", "3"))  # last-k steps with bf16 input proj
F32 = mybir.dt.float32
BF16 = mybir.dt.bfloat16
F8 = mybir.dt.float8e4
DR = mybir.MatmulPerfMode.DoubleRow

ACT = mybir.ActivationFunctionType
ALU = mybir.AluOpType

NP_BF = ml_dtypes.bfloat16
NP_F8 = ml_dtypes.float8_e4m3

_PROGRAM_CACHE = {}
LAST_RESULT = None


def _build_program(sched):
    """sched[t][d] = tuple of (w, w_prev_hidden, mw, hi) per step.
    w: step width (16-mult); mw: masked prefix width; hi: bf16 input proj."""
    ntiles = len(sched)
    Bc = S * ntiles
    nc = bacc.Bacc("TRN2", target_bir_lowering=False, debug=False,
                   num_devices=NCORES)

    x8_d = nc.dram_tensor("x8", [T, D, Bc], F8, kind="ExternalInput")
    xb_d = nc.dram_tensor("xb", [T, D, Bc], BF16, kind="ExternalInput")
    w8f_d = nc.dram_tensor("w8f", [D + H, G], F8, kind="ExternalInput")
    w8b_d = nc.dram_tensor("w8b", [D + H, G], F8, kind="ExternalInput")
    wbf_d = nc.dram_tensor("wbf", [D, G], BF16, kind="ExternalInput")
    wbb_d = nc.dram_tensor("wbb", [D, G], BF16, kind="ExternalInput")
    w1_d = nc.dram_tensor("w1", [2 * H, H], BF16, kind="ExternalInput")
    w2_d = nc.dram_tensor("w2", [H, H], BF16, kind="ExternalInput")
    bias_d = nc.dram_tensor("bias", [40, 128], F32, kind="ExternalInput")
    mf_d = nc.dram_tensor("maskzf", [8, Bc], BF16, kind="ExternalInput")
    mb_d = nc.dram_tensor("maskzb", [8, Bc], BF16, kind="ExternalInput")
    y_d = nc.dram_tensor("y", [H, Bc], F32, kind="ExternalOutput")

    with tile.TileContext(nc) as tc, ExitStack() as ctx:
        const = ctx.enter_context(tc.tile_pool(name="const", bufs=1))
        x8pool = ctx.enter_context(tc.tile_pool(name="x8", bufs=4))
        xbpool = ctx.enter_context(tc.tile_pool(name="xb", bufs=4))
        hbf = [ctx.enter_context(tc.tile_pool(name=f"hb{s}", bufs=2))
               for s in range(4)]
        hf8 = [ctx.enter_context(tc.tile_pool(name=f"h8{s}", bufs=2))
               for s in range(4)]
        hfin = ctx.enter_context(tc.tile_pool(name="hfin", bufs=4))
        gpool = ctx.enter_context(tc.tile_pool(name="g", bufs=16))
        mpool = ctx.enter_context(tc.tile_pool(name="m", bufs=2))
        opool = ctx.enter_context(tc.tile_pool(name="o", bufs=2))
        rzps = ctx.enter_context(tc.tile_pool(name="rz", bufs=4, space="PSUM"))
        xpps = ctx.enter_context(tc.tile_pool(name="xp", bufs=2, space="PSUM"))
        ghps = ctx.enter_context(tc.tile_pool(name="gh", bufs=2, space="PSUM"))

        def wtile(dram, kchunks, cols, dt, name):
            t_ = const.tile([128, kchunks, cols], dt, name=name)
            return t_, dram.rearrange("(c k) g -> k c g", k=128)

        w8f_t, w8f_s = wtile(w8f_d, 8, G, F8, "w8f")
        w8b_t, w8b_s = wtile(w8b_d, 8, G, F8, "w8b")
        wbf_t, wbf_s = wtile(wbf_d, 4, G, BF16, "wbf")
        wbb_t, wbb_s = wtile(wbb_d, 4, G, BF16, "wbb")
        w1, w1_s = wtile(w1_d, 8, H, BF16, "w1")
        w2, w2_s = wtile(w2_d, 4, H, BF16, "w2")
        w8 = [w8f_t, w8b_t]
        wbf = [wbf_t, wbb_t]
        bt = const.tile([128, 40], F32)
        nc.gpsimd.dma_start(bt[:], bias_d.rearrange("n p -> p n"))
        # DMA order = order of first use; fwd/bwd on separate queues so the
        # input-proj chunks of both directions land in ~6us each.
        for c in range(4):  # fp8 input chunks first (step 0 needs them)
            nc.scalar.dma_start(w8f_t[:, c, :], w8f_s[:, c, :])
            nc.gpsimd.dma_start(w8b_t[:, c, :], w8b_s[:, c, :])
        for c in range(4, 8):  # fp8 hidden chunks (step 1+)
            nc.scalar.dma_start(w8f_t[:, c, :], w8f_s[:, c, :])
            nc.gpsimd.dma_start(w8b_t[:, c, :], w8b_s[:, c, :])
        for c in range(4):  # bf16 input weights (last-3 steps only)
            nc.scalar.dma_start(wbf_t[:, c, :], wbf_s[:, c, :])
            nc.gpsimd.dma_start(wbb_t[:, c, :], wbb_s[:, c, :])
        for c in range(8):
            nc.scalar.dma_start(w1[:, c, :], w1_s[:, c, :])
        for c in range(4):
            nc.gpsimd.dma_start(w2[:, c, :], w2_s[:, c, :])
        mask_d = [mf_d, mb_d]

        # stream state: (h_prev_bf, h_prev_f8, prev_w)
        state = {}

        def emit_step(t, d, j, steps):
            w, wh, mw, hi = steps[j]
            n = len(steps)
            first = j == 0
            last = j == n - 1
            so = S - w
            soh = S - wh if not first else None  # hidden-proj suffix offset
            a0 = t * S + so
            bb = 16 * d
            pos = (8 - n + j) if d == 0 else (6 + n - j)
            skey = 2 * t + d

            if hi:
                xt = xbpool.tile([128, 4, S], BF16, tag="xb", name="xt")
                nc.sync.dma_start(
                    xt[:, :, so:],
                    xb_d[pos].rearrange("(c k) s -> k c s", k=128)[:, :, a0:a0 + w])
            else:
                xt = x8pool.tile([128, 4, S], F8, tag="x8", name="xt")
                nc.sync.dma_start(
                    xt[:, :, so:],
                    x8_d[pos].rearrange("(c k) s -> k c s", k=128)[:, :, a0:a0 + w])
            mt = None
            if mw:
                mt = mpool.tile([128, 64], BF16, tag="m", name="mt")
                nc.gpsimd.dma_start(
                    mt[:, :mw],
                    mask_d[d][8 - (n - j), a0:a0 + mw].partition_broadcast(128))

            if first:
                h_prev = h8_prev = None
            else:
                h_prev, h8_prev, _ = state[skey]
            hb_next = (hfin if last else hbf[skey]).tile(
                [128, 4, S], BF16, tag="hf" if last else f"h{skey}", name="hb")
            h8_next = None
            if not last:
                h8_next = hf8[skey].tile([128, 4, S], F8, tag=f"g8{skey}",
                                         name="h8")
                nw = steps[j + 1][0]
                if S - nw < so:  # zero newly exposed prefix for next step's dd
                    nc.gpsimd.memset(hb_next[:, :, S - nw:so].bitcast(F32), 0.0)

            rps, zps, xpns, ghns = [], [], [], []
            for i in range(4):
                c0 = i * 128
                r_ps = rzps.tile([128, w], F32, tag="rz", name=f"rps{i}")
                z_ps = rzps.tile([128, w], F32, tag="rz", name=f"zps{i}")
                xpn = xpps.tile([128, w], F32, tag="xp", name=f"xpn{i}")
                rps.append(r_ps); zps.append(z_ps); xpns.append(xpn)
                # input projections; last k-chunk deferred until after hidden
                # so the full-width stop lands on a full-width instruction
                if hi:
                    ww = wbf[d]
                    for k in range(4):
                        st = k == 0
                        lastk = k == 3 and first
                        nc.tensor.matmul(r_ps[:], ww[:, k, c0:c0 + 128],
                                         xt[:, k, so:], start=st, stop=lastk)
                        nc.tensor.matmul(z_ps[:], ww[:, k, H + c0:H + c0 + 128],
                                         xt[:, k, so:], start=st, stop=lastk)
                        nc.tensor.matmul(xpn[:], ww[:, k, 2 * H + c0:2 * H + c0 + 128],
                                         xt[:, k, so:], start=st, stop=k == 3)
                else:
                    ww = w8[d]
                    for p in range(2):
                        st = p == 0
                        lastk = p == 1 and first
                        ksl = slice(2 * p, 2 * p + 2)
                        nc.tensor.matmul(r_ps[:], ww[:, ksl, c0:c0 + 128],
                                         xt[:, ksl, so:], start=st, stop=lastk,
                                         perf_mode=DR)
                        nc.tensor.matmul(z_ps[:], ww[:, ksl, H + c0:H + c0 + 128],
                                         xt[:, ksl, so:], start=st, stop=lastk,
                                         perf_mode=DR)
                        nc.tensor.matmul(xpn[:], ww[:, ksl, 2 * H + c0:2 * H + c0 + 128],
                                         xt[:, ksl, so:], start=st, stop=p == 1,
                                         perf_mode=DR)
                if not first:
                    wwh = w8[d]
                    ghn = ghps.tile([128, wh], F32, tag="gh", name=f"ghn{i}")
                    ghns.append(ghn)
                    for p in range(2):
                        ksl = slice(4 + 2 * p, 4 + 2 * p + 2)
                        nc.tensor.matmul(rps[i][:, soh - so:],
                                         wwh[:, ksl, c0:c0 + 128],
                                         h8_prev[:, 2 * p:2 * p + 2, soh:],
                                         start=False, stop=p == 1, perf_mode=DR)
                        nc.tensor.matmul(zps[i][:, soh - so:],
                                         wwh[:, ksl, H + c0:H + c0 + 128],
                                         h8_prev[:, 2 * p:2 * p + 2, soh:],
                                         start=False, stop=p == 1, perf_mode=DR)
                        nc.tensor.matmul(ghn[:],
                                         wwh[:, ksl, 2 * H + c0:2 * H + c0 + 128],
                                         h8_prev[:, 2 * p:2 * p + 2, soh:],
                                         start=p == 0, stop=p == 1, perf_mode=DR)

            for i in range(4):
                if mw:
                    nc.vector.tensor_add(zps[i][:, :mw], zps[i][:, :mw],
                                         mt[:, :mw])
                r = gpool.tile([128, w], BF16, tag="g", name="r")
                nc.scalar.activation(r[:], rps[i][:], ACT.Sigmoid,
                                     bias=bt[:, bb + i:bb + i + 1],
                                     scale=1.0 / 512)
                z = gpool.tile([128, w], BF16, tag="g", name="z")
                nc.scalar.activation(z[:], zps[i][:], ACT.Sigmoid,
                                     bias=bt[:, bb + 4 + i:bb + 5 + i],
                                     scale=1.0 / 512)
                tt = gpool.tile([128, w], BF16, tag="g", name="tt")
                if first:
                    nc.vector.tensor_scalar_mul(tt[:], r[:],
                                                bt[:, bb + 8 + i:bb + 9 + i])
                else:
                    dd = soh - so
                    if dd:
                        nc.vector.tensor_scalar_mul(tt[:, :dd], r[:, :dd],
                                                    bt[:, bb + 8 + i:bb + 9 + i])
                    nc.vector.scalar_tensor_tensor(
                        tt[:, dd:], ghns[i][:], bt[:, bb + 8 + i:bb + 9 + i],
                        r[:, dd:], op0=ALU.add, op1=ALU.mult)
                ss = gpool.tile([128, w], BF16, tag="g", name="ss")
                nc.vector.tensor_add(ss[:], tt[:], xpns[i][:])
                nn = gpool.tile([128, w], BF16, tag="g", name="n")
                nc.scalar.activation(nn[:], ss[:], ACT.Tanh,
                                     bias=bt[:, bb + 12 + i:bb + 13 + i],
                                     scale=1.0 / 512)
                # critical chain (feeds next step's hidden matmuls via h8)
                # stays on vector; the bf16 carry copy goes to gpsimd.
                ho = hb_next[:, i, so:]
                if first:
                    e = gpool.tile([128, w], BF16, tag="g", name="e")
                    nc.vector.tensor_mul(e[:], z[:], nn[:])
                    if h8_next is not None:
                        nc.vector.tensor_sub(h8_next[:, i, so:], nn[:], e[:])
                        nc.gpsimd.tensor_sub(ho, nn[:], e[:])
                    else:
                        nc.vector.tensor_sub(ho, nn[:], e[:])
                else:
                    dd_t = gpool.tile([128, w], BF16, tag="g", name="dd")
                    nc.vector.tensor_sub(dd_t[:], h_prev[:, i, so:], nn[:])
                    e = gpool.tile([128, w], BF16, tag="g", name="e")
                    nc.vector.tensor_mul(e[:], z[:], dd_t[:])
                    if h8_next is not None:
                        nc.vector.tensor_add(h8_next[:, i, so:], nn[:], e[:])
                        nc.gpsimd.tensor_add(ho, nn[:], e[:])
                    else:
                        nc.vector.tensor_add(ho, nn[:], e[:])
            state[skey] = (hb_next, h8_next, w)
            return hb_next

        def emit_mlp(t, hf_t, hb_t):
            hid = []
            for i in range(4):
                ps = xpps.tile([128, S], F32, tag="xp", name="mps")
                for k in range(8):
                    src = hf_t if k < 4 else hb_t
                    nc.tensor.matmul(ps[:], w1[:, k, i * 128:(i + 1) * 128],
                                     src[:, k % 4, :], start=k == 0, stop=k == 7)
                hr = gpool.tile([128, S], BF16, tag="g", name="hr")
                nc.scalar.activation(hr[:], ps[:], ACT.Relu,
                                     bias=bt[:, 32 + i:33 + i])
                hid.append(hr)
            for i in range(4):
                ps = xpps.tile([128, S], F32, tag="xp", name="ops")
                for k in range(4):
                    nc.tensor.matmul(ps[:], w2[:, k, i * 128:(i + 1) * 128],
                                     hid[k][:], start=k == 0, stop=k == 3)
                o32 = opool.tile([128, S], F32, tag="o", name="o32")
                nc.vector.tensor_scalar_add(o32[:], ps[:], bt[:, 36 + i:37 + i])
                nc.sync.dma_start(y_d[i * 128:(i + 1) * 128, t * S:(t + 1) * S],
                                  o32[:])

        # interleave the four streams super-step by super-step (starts aligned)
        nmax = max(len(sched[t][d]) for t in range(ntiles) for d in range(2))
        hfs = {}
        mlp_done = set()
        for J in range(nmax):
            for t in range(ntiles):
                for d in range(2):
                    steps = sched[t][d]
                    if J < len(steps):
                        h = emit_step(t, d, J, steps)
                        if J == len(steps) - 1:
                            hfs[(t, d)] = h
            for t in range(ntiles):
                if t not in mlp_done and (t, 0) in hfs and (t, 1) in hfs:
                    emit_mlp(t, hfs[(t, 0)], hfs[(t, 1)])
                    mlp_done.add(t)

    nc.compile()
    return nc


def _mk_sched(lens_pc, t):
    """lens_pc: [1024, NCORES] per-core sorted lengths; tile t rows."""
    seg = lens_pc[t * S:(t + 1) * S]  # [S, NCORES]
    n = int(seg.max())
    steps = []
    for j in range(n):
        need = n - j
        cnt = (seg >= need).sum(axis=0)
        w = min(S, -(-int(cnt.max()) // 16) * 16)
        mw = int(w - int(cnt.min()))
        hi = j >= n - N_HI
        steps.append([w, 0, mw, hi])
    for j in range(1, n):
        steps[j][1] = steps[j - 1][0]  # hidden width = prev step width
    return tuple(tuple(s) for s in steps)


def kernel(padded_window, window_len, Wih_f, Whh_f, bih_f, bhh_f,
           Wih_b, Whh_b, bih_b, bhh_b, W1, b1, W2, b2):
    wl = np.asarray(window_len)
    lf = (wl - 1) // 2 + 1
    lb = wl // 2 + 1
    order = np.argsort(wl, kind="stable")

    Bc = B // NCORES
    ntiles = Bc // S
    lf_pc = lf[order].reshape(-1, NCORES)
    lb_pc = lb[order].reshape(-1, NCORES)

    sched = tuple((_mk_sched(lf_pc, t), _mk_sched(lb_pc, t))
                  for t in range(ntiles))

    if sched not in _PROGRAM_CACHE:
        _PROGRAM_CACHE[sched] = _build_program(sched)
    nc = _PROGRAM_CACHE[sched]

    f32 = np.float32
    wf_full = np.concatenate([Wih_f.T, Whh_f.T], 0).astype(f32) * 512.0
    wb_full = np.concatenate([Wih_b.T, Whh_b.T], 0).astype(f32) * 512.0
    w8f = np.clip(wf_full, -240, 240).astype(NP_F8)
    w8b = np.clip(wb_full, -240, 240).astype(NP_F8)
    wbf = wf_full[:D].astype(NP_BF)
    wbb = wb_full[:D].astype(NP_BF)
    w1 = np.ascontiguousarray(W1.T, dtype=f32).astype(NP_BF)
    w2 = np.ascontiguousarray(W2.T, dtype=f32).astype(NP_BF)

    def chunks(v):
        return np.asarray(v, f32).reshape(4, 128)

    bias = np.concatenate([
        chunks((bih_f + bhh_f)[:H]), chunks((bih_f + bhh_f)[H:2 * H]),
        chunks(bhh_f[2 * H:] * 512.0), chunks(bih_f[2 * H:]),
        chunks((bih_b + bhh_b)[:H]), chunks((bih_b + bhh_b)[H:2 * H]),
        chunks(bhh_b[2 * H:] * 512.0), chunks(bih_b[2 * H:]),
        chunks(b1), chunks(b2),
    ], 0)  # [40, 128]

    pw = np.asarray(padded_window, f32)
    in_maps = []
    p8 = np.arange(8)
    for c in range(NCORES):
        idx = order[c::NCORES]
        xT = np.ascontiguousarray(pw[idx].transpose(1, 2, 0))  # [15, 512, Bc]
        mzf = (512.0 * BIG * (p8[:, None] < (8 - lf[idx])[None, :])).astype(NP_BF)
        mzb = (512.0 * BIG * (p8[:, None] < (8 - lb[idx])[None, :])).astype(NP_BF)
        in_maps.append({
            "x8": np.clip(xT, -240, 240).astype(NP_F8),
            "xb": xT.astype(NP_BF),
            "w8f": w8f, "w8b": w8b, "wbf": wbf, "wbb": wbb,
            "w1": w1, "w2": w2, "bias": bias,
            "maskzf": mzf, "maskzb": mzb,
        })

    trace = bool(os.environ.get("GRU_TRACE"))
    kw = {}
    if os.environ.get("GRU_TMPDIR"):
        kw["tmpdir"] = os.environ["GRU_TMPDIR"]
    res = run_bass_kernel_spmd(nc, in_maps, core_ids=list(range(NCORES)),
                               trace=trace, **kw)
    global LAST_RESULT
    LAST_RESULT = res
    out = np.empty((B, H), f32)
    for c in range(NCORES):
        out[order[c::NCORES]] = res.results[c]["y"].T
    return out


# revision 11
# speedup vs baseline: 1.4698x; 1.0888x over previous
"""BiGRU encoder kernel for 8 Trainium2 NeuronCores.

Strategy (v2, fp8 DoubleRow):
  - Masked GRU over FIXED position ranges as before: forward runs positions
    (8-n)..7 ascending, backward (6+n)..7 descending; a sample of length l
    starts at step n-l with h=0 (prefix memset) and a +BIG z-gate mask keeps
    over-included samples at exactly h=0 until their true start.
  - Sort samples by window_len, deal round-robin to 8 cores; per core two
    batch tiles of 512 sorted samples.  Step widths are EXACT per-step active
    counts (max over cores), rounded up to 16 only so SBUF suffix offsets stay
    16B-aligned; the <=15+spread over-included samples are fixed by a narrow
    z-mask add.
  - Matmuls: fp8e4 DoubleRow (K=256 per instruction, measured 2x throughput)
    for ALL hidden projections and for input projections except the last
    N_HI=3 steps of each stream, which run in bf16 for accuracy (fp8 error on
    late steps flows undamped into the output).  Weights are pre-scaled by
    512 (exact power-of-2) so unscaled fp8 x/h stay in e4m3's normal range;
    every activation rescales with scale=1/512.
  - h is carried in bf16 (fp8 carry compounds error); an fp8 copy for the
    next step's matmul operand is produced by a parallel gpsimd op.
  - Hidden projections run at the PREVIOUS step's width (newly exposed
    samples have h=0 so contribute nothing); the n-gate pre-activation is
    assembled with a split tensor op at the exposure boundary.
  - All four streams (2 tiles x fwd/bwd) are interleaved super-step by
    super-step so gate latency of one stream hides under matmuls of others.
  - Output is written feature-major [H, Bc]; the host transposes (free).
"""

import os
from contextlib import ExitStack

import numpy as np
import ml_dtypes

import concourse.bacc as bacc
import concourse.tile as tile
from concourse import mybir
from concourse.bass_utils import run_bass_kernel_spmd

NCORES = 8
B, T, D, H = 8192, 15, 512, 512
G = 3 * H
BIG = 40.0
S = 512
N_HI = int(os.environ.get("GRU_NHI", "3"))  # last-k steps with bf16 input proj
F32 = mybir.dt.float32
BF16 = mybir.dt.bfloat16
F8 = mybir.dt.float8e4
DR = mybir.MatmulPerfMode.DoubleRow

ACT = mybir.ActivationFunctionType
ALU = mybir.AluOpType

NP_BF = ml_dtypes.bfloat16
NP_F8 = ml_dtypes.float8_e4m3

_PROGRAM_CACHE = {}
LAST_RESULT = None


def _build_program(sched):
    """sched[t][d] = tuple of (w, w_prev_hidden, mw, hi) per step.
    w: step width (16-mult); mw: masked prefix width; hi: bf16 input proj."""
    ntiles = len(sched)
    Bc = S * ntiles
    nc = bacc.Bacc("TRN2", target_bir_lowering=False, debug=False,
                   num_devices=NCORES)

    x8_d = nc.dram_tensor("x8", [T, D, Bc], F8, kind="ExternalInput")
    xb_d = nc.dram_tensor("xb", [T, D, Bc], BF16, kind="ExternalInput")
    w8f_d = nc.dram_tensor("w8f", [D + H, G], F8, kind="ExternalInput")
    w8b_d = nc.dram_tensor("w8b", [D + H, G], F8, kind="ExternalInput")
    wbf_d = nc.dram_tensor("wbf", [D, G], BF16, kind="ExternalInput")
    wbb_d = nc.dram_tensor("wbb", [D, G], BF16, kind="ExternalInput")
    w1_d = nc.dram_tensor("w1", [2 * H, H], BF16, kind="ExternalInput")
    w2_d = nc.dram_tensor("w2", [H, H], BF16, kind="ExternalInput")
    bias_d = nc.dram_tensor("bias", [40, 128], F32, kind="ExternalInput")
    mf_d = nc.dram_tensor("maskzf", [8, Bc], BF16, kind="ExternalInput")
    mb_d = nc.dram_tensor("maskzb", [8, Bc], BF16, kind="ExternalInput")
    y_d = nc.dram_tensor("y", [H, Bc], F32, kind="ExternalOutput")

    with tile.TileContext(nc) as tc, ExitStack() as ctx:
        const = ctx.enter_context(tc.tile_pool(name="const", bufs=1))
        x8pool = ctx.enter_context(tc.tile_pool(name="x8", bufs=4))
        xbpool = ctx.enter_context(tc.tile_pool(name="xb", bufs=4))
        hbf = [ctx.enter_context(tc.tile_pool(name=f"hb{s}", bufs=2))
               for s in range(4)]
        hf8 = [ctx.enter_context(tc.tile_pool(name=f"h8{s}", bufs=2))
               for s in range(4)]
        hfin = ctx.enter_context(tc.tile_pool(name="hfin", bufs=4))
        gpool = ctx.enter_context(tc.tile_pool(name="g", bufs=16))
        mpool = ctx.enter_context(tc.tile_pool(name="m", bufs=2))
        opool = ctx.enter_context(tc.tile_pool(name="o", bufs=2))
        rzps = ctx.enter_context(tc.tile_pool(name="rz", bufs=4, space="PSUM"))
        xpps = ctx.enter_context(tc.tile_pool(name="xp", bufs=2, space="PSUM"))
        ghps = ctx.enter_context(tc.tile_pool(name="gh", bufs=2, space="PSUM"))

        def wtile(dram, kchunks, cols, dt, name):
            t_ = const.tile([128, kchunks, cols], dt, name=name)
            return t_, dram.rearrange("(c k) g -> k c g", k=128)

        w8f_t, w8f_s = wtile(w8f_d, 8, G, F8, "w8f")
        w8b_t, w8b_s = wtile(w8b_d, 8, G, F8, "w8b")
        wbf_t, wbf_s = wtile(wbf_d, 4, G, BF16, "wbf")
        wbb_t, wbb_s = wtile(wbb_d, 4, G, BF16, "wbb")
        w1, w1_s = wtile(w1_d, 8, H, BF16, "w1")
        w2, w2_s = wtile(w2_d, 4, H, BF16, "w2")
        w8 = [w8f_t, w8b_t]
        wbf = [wbf_t, wbb_t]
        bt = const.tile([128, 40], F32)
        nc.gpsimd.dma_start(bt[:], bias_d.rearrange("n p -> p n"))
        # DMA order = order of first use.  The scalar queue must stay clear
        # early (the ACT table load + first gates run there); weights go on
        # sync/gpsimd in need-order, w1/w2 go on scalar after step 2.
        for c in range(4):  # fp8 input chunks first (step 0 needs them)
            nc.sync.dma_start(w8f_t[:, c, :], w8f_s[:, c, :])
            nc.gpsimd.dma_start(w8b_t[:, c, :], w8b_s[:, c, :])
        for c in range(4):  # bf16 input weights (short streams hit hi early)
            nc.gpsimd.dma_start(wbf_t[:, c, :], wbf_s[:, c, :])
            nc.gpsimd.dma_start(wbb_t[:, c, :], wbb_s[:, c, :])
        mask_d = [mf_d, mb_d]

        def load_late_weights_a():  # fp8 hidden chunks (needed from step 1)
            for c in range(4, 8):
                nc.sync.dma_start(w8f_t[:, c, :], w8f_s[:, c, :])
                nc.sync.dma_start(w8b_t[:, c, :], w8b_s[:, c, :])

        def load_late_weights_b():  # MLP weights (needed after short streams)
            for c in range(8):
                nc.scalar.dma_start(w1[:, c, :], w1_s[:, c, :])
            for c in range(4):
                nc.scalar.dma_start(w2[:, c, :], w2_s[:, c, :])

        # stream state: (h_prev_bf, h_prev_f8, prev_w)
        state = {}

        def emit_step(t, d, j, steps):
            w, wh, mw, hi = steps[j]
            n = len(steps)
            first = j == 0
            last = j == n - 1
            so = S - w
            soh = S - wh if not first else None  # hidden-proj suffix offset
            a0 = t * S + so
            bb = 16 * d
            pos = (8 - n + j) if d == 0 else (6 + n - j)
            skey = 2 * t + d

            if hi:
                xt = xbpool.tile([128, 4, S], BF16, tag="xb", name="xt")
                nc.sync.dma_start(
                    xt[:, :, so:],
                    xb_d[pos].rearrange("(c k) s -> k c s", k=128)[:, :, a0:a0 + w])
            else:
                xt = x8pool.tile([128, 4, S], F8, tag="x8", name="xt")
                nc.sync.dma_start(
                    xt[:, :, so:],
                    x8_d[pos].rearrange("(c k) s -> k c s", k=128)[:, :, a0:a0 + w])
            mt = None
            if mw:
                mt = mpool.tile([128, 64], BF16, tag="m", name="mt")
                nc.gpsimd.dma_start(
                    mt[:, :mw],
                    mask_d[d][8 - (n - j), a0:a0 + mw].partition_broadcast(128))

            if first:
                h_prev = h8_prev = None
            else:
                h_prev, h8_prev, _ = state[skey]
            hb_next = (hfin if last else hbf[skey]).tile(
                [128, 4, S], BF16, tag="hf" if last else f"h{skey}", name="hb")
            h8_next = None
            if not last:
                h8_next = hf8[skey].tile([128, 4, S], F8, tag=f"g8{skey}",
                                         name="h8")
                nw = steps[j + 1][0]
                if S - nw < so:  # zero newly exposed prefix for next step's dd
                    nc.gpsimd.memset(hb_next[:, :, S - nw:so].bitcast(F32), 0.0)

            rps, zps, xpns, ghns = [], [], [], []
            for i in range(4):
                c0 = i * 128
                r_ps = rzps.tile([128, w], F32, tag="rz", name=f"rps{i}")
                z_ps = rzps.tile([128, w], F32, tag="rz", name=f"zps{i}")
                xpn = xpps.tile([128, w], F32, tag="xp", name=f"xpn{i}")
                rps.append(r_ps); zps.append(z_ps); xpns.append(xpn)
                # input projections; last k-chunk deferred until after hidden
                # so the full-width stop lands on a full-width instruction
                if hi:
                    ww = wbf[d]
                    for k in range(4):
                        st = k == 0
                        lastk = k == 3 and first
                        nc.tensor.matmul(r_ps[:], ww[:, k, c0:c0 + 128],
                                         xt[:, k, so:], start=st, stop=lastk)
                        nc.tensor.matmul(z_ps[:], ww[:, k, H + c0:H + c0 + 128],
                                         xt[:, k, so:], start=st, stop=lastk)
                        nc.tensor.matmul(xpn[:], ww[:, k, 2 * H + c0:2 * H + c0 + 128],
                                         xt[:, k, so:], start=st, stop=k == 3)
                else:
                    ww = w8[d]
                    for p in range(2):
                        st = p == 0
                        lastk = p == 1 and first
                        ksl = slice(2 * p, 2 * p + 2)
                        nc.tensor.matmul(r_ps[:], ww[:, ksl, c0:c0 + 128],
                                         xt[:, ksl, so:], start=st, stop=lastk,
                                         perf_mode=DR)
                        nc.tensor.matmul(z_ps[:], ww[:, ksl, H + c0:H + c0 + 128],
                                         xt[:, ksl, so:], start=st, stop=lastk,
                                         perf_mode=DR)
                        nc.tensor.matmul(xpn[:], ww[:, ksl, 2 * H + c0:2 * H + c0 + 128],
                                         xt[:, ksl, so:], start=st, stop=p == 1,
                                         perf_mode=DR)
                if not first:
                    wwh = w8[d]
                    ghn = ghps.tile([128, wh], F32, tag="gh", name=f"ghn{i}")
                    ghns.append(ghn)
                    for p in range(2):
                        ksl = slice(4 + 2 * p, 4 + 2 * p + 2)
                        nc.tensor.matmul(rps[i][:, soh - so:],
                                         wwh[:, ksl, c0:c0 + 128],
                                         h8_prev[:, 2 * p:2 * p + 2, soh:],
                                         start=False, stop=p == 1, perf_mode=DR)
                        nc.tensor.matmul(zps[i][:, soh - so:],
                                         wwh[:, ksl, H + c0:H + c0 + 128],
                                         h8_prev[:, 2 * p:2 * p + 2, soh:],
                                         start=False, stop=p == 1, perf_mode=DR)
                        nc.tensor.matmul(ghn[:],
                                         wwh[:, ksl, 2 * H + c0:2 * H + c0 + 128],
                                         h8_prev[:, 2 * p:2 * p + 2, soh:],
                                         start=p == 0, stop=p == 1, perf_mode=DR)

            for i in range(4):
                if mw:
                    nc.vector.tensor_add(zps[i][:, :mw], zps[i][:, :mw],
                                         mt[:, :mw])
                r = gpool.tile([128, w], BF16, tag="g", name="r")
                nc.scalar.activation(r[:], rps[i][:], ACT.Sigmoid,
                                     bias=bt[:, bb + i:bb + i + 1],
                                     scale=1.0 / 512)
                z = gpool.tile([128, w], BF16, tag="g", name="z")
                nc.scalar.activation(z[:], zps[i][:], ACT.Sigmoid,
                                     bias=bt[:, bb + 4 + i:bb + 5 + i],
                                     scale=1.0 / 512)
                tt = gpool.tile([128, w], BF16, tag="g", name="tt")
                if first:
                    # tt = r * bhh_n  via ACT Copy with per-partition scale
                    nc.scalar.activation(tt[:], r[:], ACT.Copy, bias=0.0,
                                         scale=bt[:, bb + 8 + i:bb + 9 + i])
                else:
                    dd = soh - so
                    if dd:
                        nc.scalar.activation(tt[:, :dd], r[:, :dd], ACT.Copy,
                                             bias=0.0,
                                             scale=bt[:, bb + 8 + i:bb + 9 + i])
                    nc.vector.scalar_tensor_tensor(
                        tt[:, dd:], ghns[i][:], bt[:, bb + 8 + i:bb + 9 + i],
                        r[:, dd:], op0=ALU.add, op1=ALU.mult)
                ss = gpool.tile([128, w], BF16, tag="g", name="ss")
                nc.vector.tensor_add(ss[:], tt[:], xpns[i][:])
                nn = gpool.tile([128, w], BF16, tag="g", name="n")
                nc.scalar.activation(nn[:], ss[:], ACT.Tanh,
                                     bias=bt[:, bb + 12 + i:bb + 13 + i],
                                     scale=1.0 / 512)
                # critical chain (feeds next step's hidden matmuls via h8)
                # stays on vector; the bf16 carry copy goes to gpsimd.
                ho = hb_next[:, i, so:]
                if first:
                    e = gpool.tile([128, w], BF16, tag="g", name="e")
                    nc.vector.tensor_mul(e[:], z[:], nn[:])
                    if h8_next is not None:
                        nc.vector.tensor_sub(h8_next[:, i, so:], nn[:], e[:])
                        nc.gpsimd.tensor_sub(ho, nn[:], e[:])
                    else:
                        nc.vector.tensor_sub(ho, nn[:], e[:])
                else:
                    dd_t = gpool.tile([128, w], BF16, tag="g", name="dd")
                    nc.vector.tensor_sub(dd_t[:], h_prev[:, i, so:], nn[:])
                    e = gpool.tile([128, w], BF16, tag="g", name="e")
                    nc.vector.tensor_mul(e[:], z[:], dd_t[:])
                    if h8_next is not None:
                        nc.vector.tensor_add(h8_next[:, i, so:], nn[:], e[:])
                        nc.gpsimd.tensor_add(ho, nn[:], e[:])
                    else:
                        nc.vector.tensor_add(ho, nn[:], e[:])
            state[skey] = (hb_next, h8_next, w)
            return hb_next

        def emit_mlp(t, hf_t, hb_t):
            hid = []
            for i in range(4):
                ps = xpps.tile([128, S], F32, tag="xp", name="mps")
                for k in range(8):
                    src = hf_t if k < 4 else hb_t
                    nc.tensor.matmul(ps[:], w1[:, k, i * 128:(i + 1) * 128],
                                     src[:, k % 4, :], start=k == 0, stop=k == 7)
                hr = gpool.tile([128, S], BF16, tag="g", name="hr")
                nc.scalar.activation(hr[:], ps[:], ACT.Relu,
                                     bias=bt[:, 32 + i:33 + i])
                hid.append(hr)
            for i in range(4):
                ps = xpps.tile([128, S], F32, tag="xp", name="ops")
                for k in range(4):
                    nc.tensor.matmul(ps[:], w2[:, k, i * 128:(i + 1) * 128],
                                     hid[k][:], start=k == 0, stop=k == 3)
                o32 = opool.tile([128, S], F32, tag="o", name="o32")
                nc.vector.tensor_scalar_add(o32[:], ps[:], bt[:, 36 + i:37 + i])
                nc.sync.dma_start(y_d[i * 128:(i + 1) * 128, t * S:(t + 1) * S],
                                  o32[:])

        # interleave the four streams super-step by super-step (starts aligned)
        nmax = max(len(sched[t][d]) for t in range(ntiles) for d in range(2))
        hfs = {}
        mlp_done = set()
        for J in range(nmax):
            for t in range(ntiles):
                for d in range(2):
                    steps = sched[t][d]
                    if J < len(steps):
                        h = emit_step(t, d, J, steps)
                        if J == len(steps) - 1:
                            hfs[(t, d)] = h
            if J == 0:
                load_late_weights_a()
            elif J == 2:
                load_late_weights_b()
            for t in range(ntiles):
                if t not in mlp_done and (t, 0) in hfs and (t, 1) in hfs:
                    emit_mlp(t, hfs[(t, 0)], hfs[(t, 1)])
                    mlp_done.add(t)

    nc.compile()
    return nc


def _mk_sched(lens_pc, t):
    """lens_pc: [1024, NCORES] per-core sorted lengths; tile t rows."""
    seg = lens_pc[t * S:(t + 1) * S]  # [S, NCORES]
    n = int(seg.max())
    steps = []
    for j in range(n):
        need = n - j
        cnt = (seg >= need).sum(axis=0)
        w = min(S, -(-int(cnt.max()) // 16) * 16)
        mw = int(w - int(cnt.min()))
        hi = j >= n - N_HI
        steps.append([w, 0, mw, hi])
    for j in range(1, n):
        steps[j][1] = steps[j - 1][0]  # hidden width = prev step width
    return tuple(tuple(s) for s in steps)


def kernel(padded_window, window_len, Wih_f, Whh_f, bih_f, bhh_f,
           Wih_b, Whh_b, bih_b, bhh_b, W1, b1, W2, b2):
    wl = np.asarray(window_len)
    lf = (wl - 1) // 2 + 1
    lb = wl // 2 + 1
    order = np.argsort(wl, kind="stable")

    Bc = B // NCORES
    ntiles = Bc // S
    lf_pc = lf[order].reshape(-1, NCORES)
    lb_pc = lb[order].reshape(-1, NCORES)

    sched = tuple((_mk_sched(lf_pc, t), _mk_sched(lb_pc, t))
                  for t in range(ntiles))

    if sched not in _PROGRAM_CACHE:
        _PROGRAM_CACHE[sched] = _build_program(sched)
    nc = _PROGRAM_CACHE[sched]

    f32 = np.float32
    wf_full = np.concatenate([Wih_f.T, Whh_f.T], 0).astype(f32) * 512.0
    wb_full = np.concatenate([Wih_b.T, Whh_b.T], 0).astype(f32) * 512.0
    w8f = np.clip(wf_full, -240, 240).astype(NP_F8)
    w8b = np.clip(wb_full, -240, 240).astype(NP_F8)
    wbf = wf_full[:D].astype(NP_BF)
    wbb = wb_full[:D].astype(NP_BF)
    w1 = np.ascontiguousarray(W1.T, dtype=f32).astype(NP_BF)
    w2 = np.ascontiguousarray(W2.T, dtype=f32).astype(NP_BF)

    def chunks(v):
        return np.asarray(v, f32).reshape(4, 128)

    bias = np.concatenate([
        chunks((bih_f + bhh_f)[:H]), chunks((bih_f + bhh_f)[H:2 * H]),
        chunks(bhh_f[2 * H:] * 512.0), chunks(bih_f[2 * H:]),
        chunks((bih_b + bhh_b)[:H]), chunks((bih_b + bhh_b)[H:2 * H]),
        chunks(bhh_b[2 * H:] * 512.0), chunks(bih_b[2 * H:]),
        chunks(b1), chunks(b2),
    ], 0)  # [40, 128]

    pw = np.asarray(padded_window, f32)
    in_maps = []
    p8 = np.arange(8)
    for c in range(NCORES):
        idx = order[c::NCORES]
        xT = np.ascontiguousarray(pw[idx].transpose(1, 2, 0))  # [15, 512, Bc]
        mzf = (512.0 * BIG * (p8[:, None] < (8 - lf[idx])[None, :])).astype(NP_BF)
        mzb = (512.0 * BIG * (p8[:, None] < (8 - lb[idx])[None, :])).astype(NP_BF)
        in_maps.append({
            "x8": np.clip(xT, -240, 240).astype(NP_F8),
            "xb": xT.astype(NP_BF),
            "w8f": w8f, "w8b": w8b, "wbf": wbf, "wbb": wbb,
            "w1": w1, "w2": w2, "bias": bias,
            "maskzf": mzf, "maskzb": mzb,
        })

    trace = bool(os.environ.get("GRU_TRACE"))
    kw = {}
    if os.environ.get("GRU_TMPDIR"):
        kw["tmpdir"] = os.environ["GRU_TMPDIR"]
    res = run_bass_kernel_spmd(nc, in_maps, core_ids=list(range(NCORES)),
                               trace=trace, **kw)
    global LAST_RESULT
    LAST_RESULT = res
    out = np.empty((B, H), f32)
    for c in range(NCORES):
        out[order[c::NCORES]] = res.results[c]["y"].T
    return out


# revision 12
# speedup vs baseline: 1.5232x; 1.0363x over previous
"""BiGRU encoder kernel for 8 Trainium2 NeuronCores.

Strategy (v2, fp8 DoubleRow):
  - Masked GRU over FIXED position ranges as before: forward runs positions
    (8-n)..7 ascending, backward (6+n)..7 descending; a sample of length l
    starts at step n-l with h=0 (prefix memset) and a +BIG z-gate mask keeps
    over-included samples at exactly h=0 until their true start.
  - Sort samples by window_len, deal round-robin to 8 cores; per core two
    batch tiles of 512 sorted samples.  Step widths are EXACT per-step active
    counts (max over cores), rounded up to 16 only so SBUF suffix offsets stay
    16B-aligned; the <=15+spread over-included samples are fixed by a narrow
    z-mask add.
  - Matmuls: fp8e4 DoubleRow (K=256 per instruction, measured 2x throughput)
    for ALL hidden projections and for input projections except the last
    N_HI=3 steps of each stream, which run in bf16 for accuracy (fp8 error on
    late steps flows undamped into the output).  Weights are pre-scaled by
    512 (exact power-of-2) so unscaled fp8 x/h stay in e4m3's normal range;
    every activation rescales with scale=1/512.
  - h is carried in bf16 (fp8 carry compounds error); an fp8 copy for the
    next step's matmul operand is produced by a parallel gpsimd op.
  - Hidden projections run at the PREVIOUS step's width (newly exposed
    samples have h=0 so contribute nothing); the n-gate pre-activation is
    assembled with a split tensor op at the exposure boundary.
  - All four streams (2 tiles x fwd/bwd) are interleaved super-step by
    super-step so gate latency of one stream hides under matmuls of others.
  - Output is written feature-major [H, Bc]; the host transposes (free).
"""

import os
from contextlib import ExitStack

import numpy as np
import ml_dtypes

import concourse.bacc as bacc
import concourse.tile as tile
from concourse import mybir
from concourse.bass_utils import run_bass_kernel_spmd

NCORES = 8
B, T, D, H = 8192, 15, 512, 512
G = 3 * H
BIG = 40.0
S = 512
N_HI = int(os.environ.get("GRU_NHI", "3"))  # last-k steps with bf16 input proj
F32 = mybir.dt.float32
BF16 = mybir.dt.bfloat16
F8 = mybir.dt.float8e4
DR = mybir.MatmulPerfMode.DoubleRow

ACT = mybir.ActivationFunctionType
ALU = mybir.AluOpType

NP_BF = ml_dtypes.bfloat16
NP_F8 = ml_dtypes.float8_e4m3

_PROGRAM_CACHE = {}
LAST_RESULT = None


def _build_program(sched):
    """sched[t][d] = tuple of (w, w_prev_hidden, mw, hi) per step.
    w: step width (16-mult); mw: masked prefix width; hi: bf16 input proj."""
    ntiles = len(sched)
    Bc = S * ntiles
    nc = bacc.Bacc("TRN2", target_bir_lowering=False, debug=False,
                   num_devices=NCORES)

    x8_d = nc.dram_tensor("x8", [T, D, Bc], F8, kind="ExternalInput")
    xb_d = nc.dram_tensor("xb", [T, D, Bc], BF16, kind="ExternalInput")
    w8f_d = nc.dram_tensor("w8f", [D + H, G], F8, kind="ExternalInput")
    w8b_d = nc.dram_tensor("w8b", [D + H, G], F8, kind="ExternalInput")
    wbf_d = nc.dram_tensor("wbf", [D, G], BF16, kind="ExternalInput")
    wbb_d = nc.dram_tensor("wbb", [D, G], BF16, kind="ExternalInput")
    w1_d = nc.dram_tensor("w1", [2 * H, H], BF16, kind="ExternalInput")
    w2_d = nc.dram_tensor("w2", [H, H], BF16, kind="ExternalInput")
    bias_d = nc.dram_tensor("bias", [40, 128], F32, kind="ExternalInput")
    mf_d = nc.dram_tensor("maskzf", [8, Bc], BF16, kind="ExternalInput")
    mb_d = nc.dram_tensor("maskzb", [8, Bc], BF16, kind="ExternalInput")
    y_d = nc.dram_tensor("y", [H, Bc], F32, kind="ExternalOutput")

    with tile.TileContext(nc) as tc, ExitStack() as ctx:
        const = ctx.enter_context(tc.tile_pool(name="const", bufs=1))
        x8pool = ctx.enter_context(tc.tile_pool(name="x8", bufs=4))
        xbpool = ctx.enter_context(tc.tile_pool(name="xb", bufs=4))
        hbf = [ctx.enter_context(tc.tile_pool(name=f"hb{s}", bufs=2))
               for s in range(4)]
        hf8 = [ctx.enter_context(tc.tile_pool(name=f"h8{s}", bufs=2))
               for s in range(4)]
        hfin = ctx.enter_context(tc.tile_pool(name="hfin", bufs=4))
        gpool = ctx.enter_context(tc.tile_pool(name="g", bufs=16))
        mpool = ctx.enter_context(tc.tile_pool(name="m", bufs=2))
        opool = ctx.enter_context(tc.tile_pool(name="o", bufs=2))
        rzps = ctx.enter_context(tc.tile_pool(name="rz", bufs=5, space="PSUM"))
        xpps = ctx.enter_context(tc.tile_pool(name="xp", bufs=2, space="PSUM"))
        ghps = ctx.enter_context(tc.tile_pool(name="gh", bufs=1, space="PSUM"))

        def wtile(dram, kchunks, cols, dt, name):
            t_ = const.tile([128, kchunks, cols], dt, name=name)
            return t_, dram.rearrange("(c k) g -> k c g", k=128)

        w8f_t, w8f_s = wtile(w8f_d, 8, G, F8, "w8f")
        w8b_t, w8b_s = wtile(w8b_d, 8, G, F8, "w8b")
        wbf_t, wbf_s = wtile(wbf_d, 4, G, BF16, "wbf")
        wbb_t, wbb_s = wtile(wbb_d, 4, G, BF16, "wbb")
        w1, w1_s = wtile(w1_d, 8, H, BF16, "w1")
        w2, w2_s = wtile(w2_d, 4, H, BF16, "w2")
        w8 = [w8f_t, w8b_t]
        wbf = [wbf_t, wbb_t]
        bt = const.tile([128, 40], F32)
        nc.gpsimd.dma_start(bt[:], bias_d.rearrange("n p -> p n"))
        # DMA order = order of first use.  The scalar queue must stay clear
        # early (the ACT table load + first gates run there); weights go on
        # sync/gpsimd in need-order, w1/w2 go on scalar after step 2.
        for c in range(4):  # fp8 input chunks first (step 0 needs them)
            nc.sync.dma_start(w8f_t[:, c, :], w8f_s[:, c, :])
            nc.gpsimd.dma_start(w8b_t[:, c, :], w8b_s[:, c, :])
        for c in range(4):  # bf16 input weights (short streams hit hi early)
            nc.gpsimd.dma_start(wbf_t[:, c, :], wbf_s[:, c, :])
            nc.gpsimd.dma_start(wbb_t[:, c, :], wbb_s[:, c, :])
        mask_d = [mf_d, mb_d]

        def load_late_weights_a():  # fp8 hidden chunks (needed from step 1)
            for c in range(4, 8):
                nc.sync.dma_start(w8f_t[:, c, :], w8f_s[:, c, :])
                nc.sync.dma_start(w8b_t[:, c, :], w8b_s[:, c, :])

        def load_late_weights_b():  # MLP weights (needed after short streams)
            for c in range(8):
                nc.scalar.dma_start(w1[:, c, :], w1_s[:, c, :])
            for c in range(4):
                nc.scalar.dma_start(w2[:, c, :], w2_s[:, c, :])

        # stream state: (h_prev_bf, h_prev_f8, prev_w)
        state = {}

        def emit_step(t, d, j, steps):
            w, wh, mw, hi = steps[j]
            n = len(steps)
            first = j == 0
            last = j == n - 1
            so = S - w
            soh = S - wh if not first else None  # hidden-proj suffix offset
            a0 = t * S + so
            bb = 16 * d
            pos = (8 - n + j) if d == 0 else (6 + n - j)
            skey = 2 * t + d

            if hi:
                xt = xbpool.tile([128, 4, S], BF16, tag="xb", name="xt")
                nc.sync.dma_start(
                    xt[:, :, so:],
                    xb_d[pos].rearrange("(c k) s -> k c s", k=128)[:, :, a0:a0 + w])
            else:
                xt = x8pool.tile([128, 4, S], F8, tag="x8", name="xt")
                nc.sync.dma_start(
                    xt[:, :, so:],
                    x8_d[pos].rearrange("(c k) s -> k c s", k=128)[:, :, a0:a0 + w])
            mt = None
            if mw:
                mt = mpool.tile([128, 64], BF16, tag="m", name="mt")
                nc.gpsimd.dma_start(
                    mt[:, :mw],
                    mask_d[d][8 - (n - j), a0:a0 + mw].partition_broadcast(128))

            if first:
                h_prev = h8_prev = None
            else:
                h_prev, h8_prev, _ = state[skey]
            hb_next = (hfin if last else hbf[skey]).tile(
                [128, 4, S], BF16, tag="hf" if last else f"h{skey}", name="hb")
            h8_next = None
            if not last:
                h8_next = hf8[skey].tile([128, 4, S], F8, tag=f"g8{skey}",
                                         name="h8")
                nw = steps[j + 1][0]
                if S - nw < so:  # zero newly exposed prefix for next step's dd
                    nc.gpsimd.memset(hb_next[:, :, S - nw:so].bitcast(F32), 0.0)

            rps, zps, xpns, ghns = [], [], [], []
            for i in range(4):
                c0 = i * 128
                r_ps = rzps.tile([128, w], F32, tag="rz", name=f"rps{i}")
                z_ps = rzps.tile([128, w], F32, tag="rz", name=f"zps{i}")
                xpn = xpps.tile([128, w], F32, tag="xp", name=f"xpn{i}")
                rps.append(r_ps); zps.append(z_ps); xpns.append(xpn)
                # input projections; last k-chunk deferred until after hidden
                # so the full-width stop lands on a full-width instruction
                if hi:
                    ww = wbf[d]
                    for k in range(4):
                        st = k == 0
                        lastk = k == 3 and first
                        nc.tensor.matmul(r_ps[:], ww[:, k, c0:c0 + 128],
                                         xt[:, k, so:], start=st, stop=lastk)
                        nc.tensor.matmul(z_ps[:], ww[:, k, H + c0:H + c0 + 128],
                                         xt[:, k, so:], start=st, stop=lastk)
                        nc.tensor.matmul(xpn[:], ww[:, k, 2 * H + c0:2 * H + c0 + 128],
                                         xt[:, k, so:], start=st, stop=k == 3)
                else:
                    ww = w8[d]
                    for p in range(2):
                        st = p == 0
                        lastk = p == 1 and first
                        ksl = slice(2 * p, 2 * p + 2)
                        nc.tensor.matmul(r_ps[:], ww[:, ksl, c0:c0 + 128],
                                         xt[:, ksl, so:], start=st, stop=lastk,
                                         perf_mode=DR)
                        nc.tensor.matmul(z_ps[:], ww[:, ksl, H + c0:H + c0 + 128],
                                         xt[:, ksl, so:], start=st, stop=lastk,
                                         perf_mode=DR)
                        nc.tensor.matmul(xpn[:], ww[:, ksl, 2 * H + c0:2 * H + c0 + 128],
                                         xt[:, ksl, so:], start=st, stop=p == 1,
                                         perf_mode=DR)
                if not first:
                    wwh = w8[d]
                    ghn = ghps.tile([128, wh], F32, tag="gh", name=f"ghn{i}")
                    ghns.append(ghn)
                    for p in range(2):
                        ksl = slice(4 + 2 * p, 4 + 2 * p + 2)
                        nc.tensor.matmul(rps[i][:, soh - so:],
                                         wwh[:, ksl, c0:c0 + 128],
                                         h8_prev[:, 2 * p:2 * p + 2, soh:],
                                         start=False, stop=p == 1, perf_mode=DR)
                        nc.tensor.matmul(zps[i][:, soh - so:],
                                         wwh[:, ksl, H + c0:H + c0 + 128],
                                         h8_prev[:, 2 * p:2 * p + 2, soh:],
                                         start=False, stop=p == 1, perf_mode=DR)
                        nc.tensor.matmul(ghn[:],
                                         wwh[:, ksl, 2 * H + c0:2 * H + c0 + 128],
                                         h8_prev[:, 2 * p:2 * p + 2, soh:],
                                         start=p == 0, stop=p == 1, perf_mode=DR)

            for i in range(4):
                if mw:
                    nc.vector.tensor_add(zps[i][:, :mw], zps[i][:, :mw],
                                         mt[:, :mw])
                r = gpool.tile([128, w], BF16, tag="g", name="r")
                nc.scalar.activation(r[:], rps[i][:], ACT.Sigmoid,
                                     bias=bt[:, bb + i:bb + i + 1],
                                     scale=1.0 / 512)
                z = gpool.tile([128, w], BF16, tag="g", name="z")
                nc.scalar.activation(z[:], zps[i][:], ACT.Sigmoid,
                                     bias=bt[:, bb + 4 + i:bb + 5 + i],
                                     scale=1.0 / 512)
                tt = gpool.tile([128, w], BF16, tag="g", name="tt")
                if first:
                    # tt = r * bhh_n  via ACT Copy with per-partition scale
                    nc.scalar.activation(tt[:], r[:], ACT.Copy, bias=0.0,
                                         scale=bt[:, bb + 8 + i:bb + 9 + i])
                else:
                    dd = soh - so
                    if dd:
                        nc.scalar.activation(tt[:, :dd], r[:, :dd], ACT.Copy,
                                             bias=0.0,
                                             scale=bt[:, bb + 8 + i:bb + 9 + i])
                    nc.vector.scalar_tensor_tensor(
                        tt[:, dd:], ghns[i][:], bt[:, bb + 8 + i:bb + 9 + i],
                        r[:, dd:], op0=ALU.add, op1=ALU.mult)
                ss = gpool.tile([128, w], BF16, tag="g", name="ss")
                nc.vector.tensor_add(ss[:], tt[:], xpns[i][:])
                nn = gpool.tile([128, w], BF16, tag="g", name="n")
                nc.scalar.activation(nn[:], ss[:], ACT.Tanh,
                                     bias=bt[:, bb + 12 + i:bb + 13 + i],
                                     scale=1.0 / 512)
                # critical chain (feeds next step's hidden matmuls via h8)
                # stays on vector; the bf16 carry copy goes to gpsimd.
                ho = hb_next[:, i, so:]
                if first:
                    e = gpool.tile([128, w], BF16, tag="g", name="e")
                    nc.vector.tensor_mul(e[:], z[:], nn[:])
                    if h8_next is not None:
                        nc.vector.tensor_sub(h8_next[:, i, so:], nn[:], e[:])
                        nc.gpsimd.tensor_sub(ho, nn[:], e[:])
                    else:
                        nc.vector.tensor_sub(ho, nn[:], e[:])
                else:
                    dd_t = gpool.tile([128, w], BF16, tag="g", name="dd")
                    nc.vector.tensor_sub(dd_t[:], h_prev[:, i, so:], nn[:])
                    e = gpool.tile([128, w], BF16, tag="g", name="e")
                    nc.vector.tensor_mul(e[:], z[:], dd_t[:])
                    if h8_next is not None:
                        nc.vector.tensor_add(h8_next[:, i, so:], nn[:], e[:])
                        nc.gpsimd.tensor_add(ho, nn[:], e[:])
                    else:
                        nc.vector.tensor_add(ho, nn[:], e[:])
            state[skey] = (hb_next, h8_next, w)
            return hb_next

        def emit_mlp(t, hf_t, hb_t):
            hid = []
            for i in range(4):
                ps = xpps.tile([128, S], F32, tag="xp", name="mps")
                for k in range(8):
                    src = hf_t if k < 4 else hb_t
                    nc.tensor.matmul(ps[:], w1[:, k, i * 128:(i + 1) * 128],
                                     src[:, k % 4, :], start=k == 0, stop=k == 7)
                hr = gpool.tile([128, S], BF16, tag="g", name="hr")
                nc.scalar.activation(hr[:], ps[:], ACT.Relu,
                                     bias=bt[:, 32 + i:33 + i])
                hid.append(hr)
            for i in range(4):
                ps = xpps.tile([128, S], F32, tag="xp", name="ops")
                for k in range(4):
                    nc.tensor.matmul(ps[:], w2[:, k, i * 128:(i + 1) * 128],
                                     hid[k][:], start=k == 0, stop=k == 3)
                o32 = opool.tile([128, S], F32, tag="o", name="o32")
                nc.vector.tensor_scalar_add(o32[:], ps[:], bt[:, 36 + i:37 + i])
                nc.sync.dma_start(y_d[i * 128:(i + 1) * 128, t * S:(t + 1) * S],
                                  o32[:])

        # interleave the four streams super-step by super-step (starts aligned)
        nmax = max(len(sched[t][d]) for t in range(ntiles) for d in range(2))
        hfs = {}
        mlp_done = set()
        for J in range(nmax):
            for t in range(ntiles):
                for d in range(2):
                    steps = sched[t][d]
                    if J < len(steps):
                        h = emit_step(t, d, J, steps)
                        if J == len(steps) - 1:
                            hfs[(t, d)] = h
            if J == 0:
                load_late_weights_a()
            elif J == 2:
                load_late_weights_b()
            for t in range(ntiles):
                if t not in mlp_done and (t, 0) in hfs and (t, 1) in hfs:
                    emit_mlp(t, hfs[(t, 0)], hfs[(t, 1)])
                    mlp_done.add(t)

    nc.compile()
    return nc


def _mk_sched(lens_pc, t):
    """lens_pc: [1024, NCORES] per-core sorted lengths; tile t rows."""
    seg = lens_pc[t * S:(t + 1) * S]  # [S, NCORES]
    n = int(seg.max())
    steps = []
    for j in range(n):
        need = n - j
        cnt = (seg >= need).sum(axis=0)
        w = min(S, -(-int(cnt.max()) // 16) * 16)
        mw = int(w - int(cnt.min()))
        hi = j >= n - N_HI
        steps.append([w, 0, mw, hi])
    for j in range(1, n):
        steps[j][1] = steps[j - 1][0]  # hidden width = prev step width
    return tuple(tuple(s) for s in steps)


def kernel(padded_window, window_len, Wih_f, Whh_f, bih_f, bhh_f,
           Wih_b, Whh_b, bih_b, bhh_b, W1, b1, W2, b2):
    wl = np.asarray(window_len)
    lf = (wl - 1) // 2 + 1
    lb = wl // 2 + 1
    order = np.argsort(wl, kind="stable")

    Bc = B // NCORES
    ntiles = Bc // S
    lf_pc = lf[order].reshape(-1, NCORES)
    lb_pc = lb[order].reshape(-1, NCORES)

    sched = tuple((_mk_sched(lf_pc, t), _mk_sched(lb_pc, t))
                  for t in range(ntiles))

    if sched not in _PROGRAM_CACHE:
        _PROGRAM_CACHE[sched] = _build_program(sched)
    nc = _PROGRAM_CACHE[sched]

    f32 = np.float32
    wf_full = np.concatenate([Wih_f.T, Whh_f.T], 0).astype(f32) * 512.0
    wb_full = np.concatenate([Wih_b.T, Whh_b.T], 0).astype(f32) * 512.0
    w8f = np.clip(wf_full, -240, 240).astype(NP_F8)
    w8b = np.clip(wb_full, -240, 240).astype(NP_F8)
    wbf = wf_full[:D].astype(NP_BF)
    wbb = wb_full[:D].astype(NP_BF)
    w1 = np.ascontiguousarray(W1.T, dtype=f32).astype(NP_BF)
    w2 = np.ascontiguousarray(W2.T, dtype=f32).astype(NP_BF)

    def chunks(v):
        return np.asarray(v, f32).reshape(4, 128)

    bias = np.concatenate([
        chunks((bih_f + bhh_f)[:H]), chunks((bih_f + bhh_f)[H:2 * H]),
        chunks(bhh_f[2 * H:] * 512.0), chunks(bih_f[2 * H:]),
        chunks((bih_b + bhh_b)[:H]), chunks((bih_b + bhh_b)[H:2 * H]),
        chunks(bhh_b[2 * H:] * 512.0), chunks(bih_b[2 * H:]),
        chunks(b1), chunks(b2),
    ], 0)  # [40, 128]

    pw = np.asarray(padded_window, f32)
    in_maps = []
    p8 = np.arange(8)
    for c in range(NCORES):
        idx = order[c::NCORES]
        xT = np.ascontiguousarray(pw[idx].transpose(1, 2, 0))  # [15, 512, Bc]
        mzf = (512.0 * BIG * (p8[:, None] < (8 - lf[idx])[None, :])).astype(NP_BF)
        mzb = (512.0 * BIG * (p8[:, None] < (8 - lb[idx])[None, :])).astype(NP_BF)
        in_maps.append({
            "x8": np.clip(xT, -240, 240).astype(NP_F8),
            "xb": xT.astype(NP_BF),
            "w8f": w8f, "w8b": w8b, "wbf": wbf, "wbb": wbb,
            "w1": w1, "w2": w2, "bias": bias,
            "maskzf": mzf, "maskzb": mzb,
        })

    trace = bool(os.environ.get("GRU_TRACE"))
    kw = {}
    if os.environ.get("GRU_TMPDIR"):
        kw["tmpdir"] = os.environ["GRU_TMPDIR"]
    res = run_bass_kernel_spmd(nc, in_maps, core_ids=list(range(NCORES)),
                               trace=trace, **kw)
    global LAST_RESULT
    LAST_RESULT = res
    out = np.empty((B, H), f32)
    for c in range(NCORES):
        out[order[c::NCORES]] = res.results[c]["y"].T
    return out
